# revision 9
# baseline (speedup 1.0000x reference)
"""GroupGRUCell with shared schema-pool parameters — Trainium2 Bass kernel.

Problem shapes (hardcoded): B=256 batch, U=64 GRU units, DIN=H=256, S=8 schemas.
  Wx[u] = sum_s sw_x[u,s] * pool_x[s].T   (per-unit weights from shared pool)
  gate_x = x @ Wx ; gate_h = h @ Wh ; standard GRU cell gate math.

Sharding: unit-parallel across 8 NeuronCores (8 units per core); the schema
pool is replicated per core. Per core:
  - weight combine: ACT does the s=0 scaled copy, DVE (+ a GPSIMD share) does
    the remaining 7 schema MACs as scalar_tensor_tensor in bf16 (2x mode).
  - PE does the per-unit matmuls in bf16, accumulating x- and h-contributions
    for the r/i gates into the same PSUM region (i_r+h_r fused for free).
  - ACT does sigmoid/tanh, DVE the remaining elementwise gate math in f32.
"""

import numpy as np
import ml_dtypes

B, U, DIN, H, S = 256, 64, 256, 256, 8
NCORES = 8
UC = U // NCORES  # units per core
O3 = 3 * H        # 768
KC = DIN // 128   # 2 contraction chunks
MC = B // 128     # 2 batch chunks

BF16 = ml_dtypes.bfloat16

# Every Nth schema-MAC goes to GPSIMD to offload the (bottleneck) DVE.
# (10**9 = disabled: walrus rejects POOL-engine STT with many sync waits.)
GPSIMD_EVERY = 10**9


def _build_program():
    from contextlib import ExitStack

    import concourse.bacc as bacc
    import concourse.bass as bass
    import concourse.mybir as mybir
    import concourse.tile as tile

    bf = mybir.dt.bfloat16
    f32 = mybir.dt.float32
    AF = mybir.ActivationFunctionType
    ALU = mybir.AluOpType

    nc = bacc.Bacc("TRN2", target_bir_lowering=False, debug=False)

    xT = nc.dram_tensor("xT", [UC, 128, KC, B], bf, kind="ExternalInput")
    hT = nc.dram_tensor("hT", [UC, 128, KC, B], bf, kind="ExternalInput")
    hbh = nc.dram_tensor("hbh", [UC, MC, 128, H], f32, kind="ExternalInput")
    poolx = nc.dram_tensor("poolx", [S, 128, KC, O3], bf, kind="ExternalInput")
    poolh = nc.dram_tensor("poolh", [S, 128, KC, O3], bf, kind="ExternalInput")
    swx = nc.dram_tensor("swx", [128, UC * S], f32, kind="ExternalInput")
    swh = nc.dram_tensor("swh", [128, UC * S], f32, kind="ExternalInput")
    hy = nc.dram_tensor("hy", [UC, MC, 128, H], f32, kind="ExternalOutput")

    stt_ct = 0  # round-robin counter for the DVE/GPSIMD combine split

    with tile.TileContext(nc) as tc, ExitStack() as ctx:
        pconst = ctx.enter_context(tc.tile_pool(name="pconst", bufs=1))
        pwacc = ctx.enter_context(tc.tile_pool(name="pwacc", bufs=4))
        pxin = ctx.enter_context(tc.tile_pool(name="pxin", bufs=3))
        pgtmp = ctx.enter_context(tc.tile_pool(name="pgtmp", bufs=6))
        pout = ctx.enter_context(tc.tile_pool(name="pout", bufs=4))
        ppsum = ctx.enter_context(tc.tile_pool(name="ppsum", bufs=2, space="PSUM"))

        # Schema pool + mixing weights resident in SBUF for the whole kernel.
        px_sb, ph_sb = [], []
        for s in range(S):
            t = pconst.tile([128, KC, O3], bf, tag=f"poolx{s}")
            nc.sync.dma_start(out=t, in_=poolx[s])
            px_sb.append(t)
        for s in range(S):
            t = pconst.tile([128, KC, O3], bf, tag=f"poolh{s}")
            nc.sync.dma_start(out=t, in_=poolh[s])
            ph_sb.append(t)
        swx_sb = pconst.tile([128, UC * S], f32, tag="swx")
        nc.sync.dma_start(out=swx_sb, in_=swx[:, :])
        swh_sb = pconst.tile([128, UC * S], f32, tag="swh")
        nc.sync.dma_start(out=swh_sb, in_=swh[:, :])
        # Pre-touch every constant tile on DVE (and the ones ACT reads on ACT):
        # the walrus STT/ACT instruction structs encode only ONE sync wait, and
        # the combine chain already consumes it with a same-proc DVE wait, so
        # no combine op may additionally wait on a DMA proc. These tiny copies
        # make DVE/ACT observe all constant-DMA procs up front.
        def _head(t):
            return t[:, 0, 0:1] if len(t.shape) == 3 else t[:, 0:1]

        consts = px_sb + ph_sb + [swx_sb, swh_sb]
        for i, t in enumerate(consts):
            sc = pconst.tile([128, 1], f32, tag=f"scr{i}")
            nc.vector.tensor_copy(out=sc, in_=_head(t))
        for i, t in enumerate([px_sb[0], ph_sb[0], swx_sb, swh_sb]):
            sc = pconst.tile([128, 1], f32, tag=f"scra{i}")
            nc.scalar.activation(out=sc, in_=_head(t), func=AF.Copy)

        for u in range(UC):
            xt = pxin.tile([128, KC, B], bf, tag="xt")
            nc.sync.dma_start(out=xt, in_=xT[u])
            ht = pxin.tile([128, KC, B], bf, tag="ht")
            nc.sync.dma_start(out=ht, in_=hT[u])

            # --- weight combine: w = sum_s sw[u,s] * pool[s], bf16 ---
            wx = pwacc.tile([128, KC, O3], bf, tag="wx")
            wh = pwacc.tile([128, KC, O3], bf, tag="wh")
            for wt, psb, swsb in ((wx, px_sb, swx_sb), (wh, ph_sb, swh_sb)):
                col = u * S
                nc.scalar.activation(
                    out=wt,
                    in_=psb[0],
                    func=AF.Copy,
                    scale=swsb[:, col : col + 1],
                )
                for s in range(1, S):
                    eng = (
                        nc.gpsimd
                        if (stt_ct % GPSIMD_EVERY == GPSIMD_EVERY - 1)
                        else nc.vector
                    )
                    eng.scalar_tensor_tensor(
                        out=wt,
                        in0=psb[s],
                        scalar=swsb[:, col + s : col + s + 1],
                        in1=wt,
                        op0=ALU.mult,
                        op1=ALU.add,
                    )
                    stt_ct += 1

            for mc in range(MC):
                hbt = pxin.tile([128, H], f32, tag="hbt")
                nc.sync.dma_start(out=hbt, in_=hbh[u, mc])

                p_ri = ppsum.tile([128, 512], f32, tag="ri")
                p_nx = ppsum.tile([128, H], f32, tag="nx")
                p_nh = ppsum.tile([128, H], f32, tag="nh")
                bs = slice(mc * 128, (mc + 1) * 128)
                for kc in range(KC):
                    lx = xt[:, kc, bs]
                    nc.tensor.matmul(
                        p_ri, lx, wx[:, kc, 0:512], start=(kc == 0), stop=False
                    )
                    nc.tensor.matmul(
                        p_nx, lx, wx[:, kc, 512:O3], start=(kc == 0), stop=(kc == 1)
                    )
                for kc in range(KC):
                    lh = ht[:, kc, bs]
                    nc.tensor.matmul(
                        p_ri, lh, wh[:, kc, 0:512], start=False, stop=(kc == 1)
                    )
                    nc.tensor.matmul(
                        p_nh, lh, wh[:, kc, 512:O3], start=(kc == 0), stop=(kc == 1)
                    )

                # --- gate math ---
                # p_ri = [i_r + h_r | i_i + h_i]; sig = sigmoid(p_ri)
                sig = pgtmp.tile([128, 512], f32, tag="sig")
                nc.scalar.activation(out=sig, in_=p_ri, func=AF.Sigmoid)
                t1 = pgtmp.tile([128, H], f32, tag="t1")
                nc.vector.tensor_tensor(
                    out=t1, in0=sig[:, 0:H], in1=p_nh, op=ALU.mult
                )
                t2 = pgtmp.tile([128, H], f32, tag="t2")
                nc.vector.tensor_tensor(out=t2, in0=t1, in1=p_nx, op=ALU.add)
                ng = pgtmp.tile([128, H], f32, tag="ng")
                nc.scalar.activation(out=ng, in_=t2, func=AF.Tanh)
                d = pgtmp.tile([128, H], f32, tag="d")
                nc.vector.tensor_tensor(out=d, in0=hbt, in1=ng, op=ALU.subtract)
                e = pgtmp.tile([128, H], f32, tag="e")
                nc.vector.tensor_tensor(out=e, in0=sig[:, H:512], in1=d, op=ALU.mult)
                o = pout.tile([128, H], f32, tag="o")
                nc.vector.tensor_tensor(out=o, in0=ng, in1=e, op=ALU.add)
                nc.sync.dma_start(out=hy[u, mc], in_=o)

    nc.compile()
    return nc


def _prep_inputs(x, hidden, pool_x, pool_h, sw_x, sw_h):
    """Host-side (free) slicing / transposition / casting per core."""
    # pool[s, o, d] -> [s, d, o] -> [s, dp, kc, o]  (d = kc*128 + dp)
    def prep_pool(p):
        pt = np.ascontiguousarray(p.transpose(0, 2, 1))  # [S, DIN, O3]
        pt = pt.reshape(S, KC, 128, O3).transpose(0, 2, 1, 3)  # [s, dp, kc, o]
        return np.ascontiguousarray(pt.astype(BF16))

    poolx_h = prep_pool(pool_x)
    poolh_h = prep_pool(pool_h)

    in_maps = []
    for c in range(NCORES):
        us = slice(c * UC, (c + 1) * UC)
        xc = x[:, us, :]  # [B, UC, DIN]
        hc = hidden[:, us, :]
        xT_h = np.ascontiguousarray(
            xc.transpose(1, 2, 0).reshape(UC, KC, 128, B).transpose(0, 2, 1, 3).astype(BF16)
        )
        hT_h = np.ascontiguousarray(
            hc.transpose(1, 2, 0).reshape(UC, KC, 128, B).transpose(0, 2, 1, 3).astype(BF16)
        )
        hbh_h = np.ascontiguousarray(
            hc.transpose(1, 0, 2).reshape(UC, MC, 128, H).astype(np.float32)
        )
        swx_h = np.ascontiguousarray(
            np.broadcast_to(
                sw_x[us].reshape(1, UC * S).astype(np.float32), (128, UC * S)
            )
        )
        swh_h = np.ascontiguousarray(
            np.broadcast_to(
                sw_h[us].reshape(1, UC * S).astype(np.float32), (128, UC * S)
            )
        )
        in_maps.append(
            {
                "xT": xT_h,
                "hT": hT_h,
                "hbh": hbh_h,
                "poolx": poolx_h,
                "poolh": poolh_h,
                "swx": swx_h,
                "swh": swh_h,
            }
        )
    return in_maps


_CACHED_NC = None


def _get_nc():
    global _CACHED_NC
    if _CACHED_NC is None:
        _CACHED_NC = _build_program()
    return _CACHED_NC


def kernel(x, hidden, pool_x, pool_h, sw_x, sw_h, _trace=False, _results_holder=None):
    from concourse.bass_utils import run_bass_kernel_spmd

    x = np.asarray(x)
    hidden = np.asarray(hidden)
    pool_x = np.asarray(pool_x)
    pool_h = np.asarray(pool_h)
    sw_x = np.asarray(sw_x)
    sw_h = np.asarray(sw_h)

    nc = _get_nc()
    in_maps = _prep_inputs(x, hidden, pool_x, pool_h, sw_x, sw_h)
    res = run_bass_kernel_spmd(
        nc, in_maps, core_ids=list(range(NCORES)), trace=_trace
    )
    if _results_holder is not None:
        _results_holder.append(res)

    out = np.empty((B, U, H), dtype=np.float32)
    for c in range(NCORES):
        hy_c = np.asarray(res.results[c]["hy"], dtype=np.float32)  # [UC, MC, 128, H]
        out[:, c * UC : (c + 1) * UC, :] = hy_c.reshape(UC, B, H).transpose(1, 0, 2)
    return out


# revision 13
# speedup vs baseline: 1.1825x; 1.1825x over previous
"""GroupGRUCell with shared schema-pool parameters — Trainium2 Bass kernel.

Problem shapes (hardcoded): B=256 batch, U=64 GRU units, DIN=H=256, S=8 schemas.
  Wx[u] = sum_s sw_x[u,s] * pool_x[s].T   (per-unit weights from shared pool)
  gate_x = x @ Wx ; gate_h = h @ Wh ; standard GRU cell gate math.

Sharding: unit-parallel across 8 NeuronCores (8 units per core); the schema
pool is replicated per core. Per core:
  - weight combine: ACT does the s=0 scaled copy, DVE (+ a GPSIMD share) does
    the remaining 7 schema MACs as scalar_tensor_tensor in bf16 (2x mode).
  - PE does the per-unit matmuls in bf16, accumulating x- and h-contributions
    for the r/i gates into the same PSUM region (i_r+h_r fused for free).
  - ACT does sigmoid/tanh, DVE the remaining elementwise gate math in f32.
"""

import numpy as np
import ml_dtypes

B, U, DIN, H, S = 256, 64, 256, 256, 8
NCORES = 8
UC = U // NCORES  # units per core
O3 = 3 * H        # 768
KC = DIN // 128   # 2 contraction chunks
MC = B // 128     # 2 batch chunks

BF16 = ml_dtypes.bfloat16

# Every Nth schema-MAC goes to GPSIMD to offload the (bottleneck) DVE.
# (10**9 = disabled: walrus rejects POOL-engine STT with many sync waits.)
GPSIMD_EVERY = 10**9
# Split DVE schema-MACs into tensor_scalar(mul, 4x mode) + tensor_tensor(add,
# 2x mode) instead of 1x scalar_tensor_tensor.
DVE_PAIR = True
# Combine chains (one per (unit, gate), 16 per core) assigned to GPSIMD.
GPS_CHAINS = frozenset()


def _build_program():
    from contextlib import ExitStack

    import concourse.bacc as bacc
    import concourse.bass as bass
    import concourse.mybir as mybir
    import concourse.tile as tile

    bf = mybir.dt.bfloat16
    f32 = mybir.dt.float32
    AF = mybir.ActivationFunctionType
    ALU = mybir.AluOpType

    nc = bacc.Bacc("TRN2", target_bir_lowering=False, debug=False)

    xT = nc.dram_tensor("xT", [UC, 128, KC, B], bf, kind="ExternalInput")
    hT = nc.dram_tensor("hT", [UC, 128, KC, B], bf, kind="ExternalInput")
    hbh = nc.dram_tensor("hbh", [UC, MC, 128, H], f32, kind="ExternalInput")
    poolx = nc.dram_tensor("poolx", [S, 128, KC, O3], bf, kind="ExternalInput")
    poolh = nc.dram_tensor("poolh", [S, 128, KC, O3], bf, kind="ExternalInput")
    swx = nc.dram_tensor("swx", [128, UC * S], f32, kind="ExternalInput")
    swh = nc.dram_tensor("swh", [128, UC * S], f32, kind="ExternalInput")
    hy = nc.dram_tensor("hy", [UC, MC, 128, H], f32, kind="ExternalOutput")

    stt_ct = 0  # round-robin counter for the DVE/GPSIMD combine split

    with tile.TileContext(nc) as tc, ExitStack() as ctx:
        pconst = ctx.enter_context(tc.tile_pool(name="pconst", bufs=1))
        pwacc = ctx.enter_context(tc.tile_pool(name="pwacc", bufs=4))
        pxin = ctx.enter_context(tc.tile_pool(name="pxin", bufs=3))
        pgtmp = ctx.enter_context(tc.tile_pool(name="pgtmp", bufs=6))
        pout = ctx.enter_context(tc.tile_pool(name="pout", bufs=4))
        ppsum = ctx.enter_context(tc.tile_pool(name="ppsum", bufs=2, space="PSUM"))

        # Schema pool + mixing weights resident in SBUF for the whole kernel.
        px_sb, ph_sb = [], []
        for s in range(S):
            t = pconst.tile([128, KC, O3], bf, tag=f"poolx{s}")
            nc.sync.dma_start(out=t, in_=poolx[s])
            px_sb.append(t)
        for s in range(S):
            t = pconst.tile([128, KC, O3], bf, tag=f"poolh{s}")
            nc.sync.dma_start(out=t, in_=poolh[s])
            ph_sb.append(t)
        swx_sb = pconst.tile([128, UC * S], f32, tag="swx")
        nc.sync.dma_start(out=swx_sb, in_=swx[:, :])
        swh_sb = pconst.tile([128, UC * S], f32, tag="swh")
        nc.sync.dma_start(out=swh_sb, in_=swh[:, :])
        # Pre-touch every constant tile on DVE (and the ones ACT reads on ACT):
        # the walrus STT/ACT instruction structs encode only ONE sync wait, and
        # the combine chain already consumes it with a same-proc DVE wait, so
        # no combine op may additionally wait on a DMA proc. These tiny copies
        # make DVE/ACT observe all constant-DMA procs up front.
        def _head(t):
            return t[:, 0, 0:1] if len(t.shape) == 3 else t[:, 0:1]

        consts = px_sb + ph_sb + [swx_sb, swh_sb]
        for i, t in enumerate(consts):
            sc = pconst.tile([128, 1], f32, tag=f"scr{i}")
            nc.vector.tensor_copy(out=sc, in_=_head(t))
        for i, t in enumerate([px_sb[0], ph_sb[0], swx_sb, swh_sb]):
            sc = pconst.tile([128, 1], f32, tag=f"scra{i}")
            nc.scalar.activation(out=sc, in_=_head(t), func=AF.Copy)

        for u in range(UC):
            xt = pxin.tile([128, KC, B], bf, tag="xt")
            nc.sync.dma_start(out=xt, in_=xT[u])
            ht = pxin.tile([128, KC, B], bf, tag="ht")
            nc.sync.dma_start(out=ht, in_=hT[u])

            # --- weight combine: w = sum_s sw[u,s] * pool[s], bf16 ---
            wx = pwacc.tile([128, KC, O3], bf, tag="wx")
            wh = pwacc.tile([128, KC, O3], bf, tag="wh")
            for wt, psb, swsb in ((wx, px_sb, swx_sb), (wh, ph_sb, swh_sb)):
                col = u * S
                use_gps = stt_ct in GPS_CHAINS
                nc.scalar.activation(
                    out=wt,
                    in_=psb[0],
                    func=AF.Copy,
                    scale=swsb[:, col : col + 1],
                )
                for s in range(1, S):
                    if use_gps:
                        nc.gpsimd.scalar_tensor_tensor(
                            out=wt,
                            in0=psb[s],
                            scalar=swsb[:, col + s : col + s + 1],
                            in1=wt,
                            op0=ALU.mult,
                            op1=ALU.add,
                        )
                    elif DVE_PAIR:
                        tmp = pwacc.tile([128, KC, O3], bf, tag="tmp")
                        nc.vector.tensor_scalar(
                            out=tmp,
                            in0=psb[s],
                            scalar1=swsb[:, col + s : col + s + 1],
                            scalar2=None,
                            op0=ALU.mult,
                        )
                        nc.vector.tensor_tensor(
                            out=wt, in0=tmp, in1=wt, op=ALU.add
                        )
                    else:
                        nc.vector.scalar_tensor_tensor(
                            out=wt,
                            in0=psb[s],
                            scalar=swsb[:, col + s : col + s + 1],
                            in1=wt,
                            op0=ALU.mult,
                            op1=ALU.add,
                        )
                stt_ct += 1

            for mc in range(MC):
                hbt = pxin.tile([128, H], f32, tag="hbt")
                nc.sync.dma_start(out=hbt, in_=hbh[u, mc])

                p_ri = ppsum.tile([128, 512], f32, tag="ri")
                p_nx = ppsum.tile([128, H], f32, tag="nx")
                p_nh = ppsum.tile([128, H], f32, tag="nh")
                bs = slice(mc * 128, (mc + 1) * 128)
                for kc in range(KC):
                    lx = xt[:, kc, bs]
                    nc.tensor.matmul(
                        p_ri, lx, wx[:, kc, 0:512], start=(kc == 0), stop=False
                    )
                    nc.tensor.matmul(
                        p_nx, lx, wx[:, kc, 512:O3], start=(kc == 0), stop=(kc == 1)
                    )
                for kc in range(KC):
                    lh = ht[:, kc, bs]
                    nc.tensor.matmul(
                        p_ri, lh, wh[:, kc, 0:512], start=False, stop=(kc == 1)
                    )
                    nc.tensor.matmul(
                        p_nh, lh, wh[:, kc, 512:O3], start=(kc == 0), stop=(kc == 1)
                    )

                # --- gate math ---
                # p_ri = [i_r + h_r | i_i + h_i]; sig = sigmoid(p_ri)
                sig = pgtmp.tile([128, 512], f32, tag="sig")
                nc.scalar.activation(out=sig, in_=p_ri, func=AF.Sigmoid)
                t1 = pgtmp.tile([128, H], f32, tag="t1")
                nc.vector.tensor_tensor(
                    out=t1, in0=sig[:, 0:H], in1=p_nh, op=ALU.mult
                )
                t2 = pgtmp.tile([128, H], f32, tag="t2")
                nc.vector.tensor_tensor(out=t2, in0=t1, in1=p_nx, op=ALU.add)
                ng = pgtmp.tile([128, H], f32, tag="ng")
                nc.scalar.activation(out=ng, in_=t2, func=AF.Tanh)
                d = pgtmp.tile([128, H], f32, tag="d")
                nc.vector.tensor_tensor(out=d, in0=hbt, in1=ng, op=ALU.subtract)
                e = pgtmp.tile([128, H], f32, tag="e")
                nc.vector.tensor_tensor(out=e, in0=sig[:, H:512], in1=d, op=ALU.mult)
                o = pout.tile([128, H], f32, tag="o")
                nc.vector.tensor_tensor(out=o, in0=ng, in1=e, op=ALU.add)
                nc.sync.dma_start(out=hy[u, mc], in_=o)

    nc.compile()
    return nc


def _prep_inputs(x, hidden, pool_x, pool_h, sw_x, sw_h):
    """Host-side (free) slicing / transposition / casting per core."""
    # pool[s, o, d] -> [s, d, o] -> [s, dp, kc, o]  (d = kc*128 + dp)
    def prep_pool(p):
        pt = np.ascontiguousarray(p.transpose(0, 2, 1))  # [S, DIN, O3]
        pt = pt.reshape(S, KC, 128, O3).transpose(0, 2, 1, 3)  # [s, dp, kc, o]
        return np.ascontiguousarray(pt.astype(BF16))

    poolx_h = prep_pool(pool_x)
    poolh_h = prep_pool(pool_h)

    in_maps = []
    for c in range(NCORES):
        us = slice(c * UC, (c + 1) * UC)
        xc = x[:, us, :]  # [B, UC, DIN]
        hc = hidden[:, us, :]
        xT_h = np.ascontiguousarray(
            xc.transpose(1, 2, 0).reshape(UC, KC, 128, B).transpose(0, 2, 1, 3).astype(BF16)
        )
        hT_h = np.ascontiguousarray(
            hc.transpose(1, 2, 0).reshape(UC, KC, 128, B).transpose(0, 2, 1, 3).astype(BF16)
        )
        hbh_h = np.ascontiguousarray(
            hc.transpose(1, 0, 2).reshape(UC, MC, 128, H).astype(np.float32)
        )
        swx_h = np.ascontiguousarray(
            np.broadcast_to(
                sw_x[us].reshape(1, UC * S).astype(np.float32), (128, UC * S)
            )
        )
        swh_h = np.ascontiguousarray(
            np.broadcast_to(
                sw_h[us].reshape(1, UC * S).astype(np.float32), (128, UC * S)
            )
        )
        in_maps.append(
            {
                "xT": xT_h,
                "hT": hT_h,
                "hbh": hbh_h,
                "poolx": poolx_h,
                "poolh": poolh_h,
                "swx": swx_h,
                "swh": swh_h,
            }
        )
    return in_maps


_CACHED_NC = None


def _get_nc():
    global _CACHED_NC
    if _CACHED_NC is None:
        _CACHED_NC = _build_program()
    return _CACHED_NC


def kernel(x, hidden, pool_x, pool_h, sw_x, sw_h, _trace=False, _results_holder=None):
    from concourse.bass_utils import run_bass_kernel_spmd

    x = np.asarray(x)
    hidden = np.asarray(hidden)
    pool_x = np.asarray(pool_x)
    pool_h = np.asarray(pool_h)
    sw_x = np.asarray(sw_x)
    sw_h = np.asarray(sw_h)

    nc = _get_nc()
    in_maps = _prep_inputs(x, hidden, pool_x, pool_h, sw_x, sw_h)
    res = run_bass_kernel_spmd(
        nc, in_maps, core_ids=list(range(NCORES)), trace=_trace
    )
    if _results_holder is not None:
        _results_holder.append(res)

    out = np.empty((B, U, H), dtype=np.float32)
    for c in range(NCORES):
        hy_c = np.asarray(res.results[c]["hy"], dtype=np.float32)  # [UC, MC, 128, H]
        out[:, c * UC : (c + 1) * UC, :] = hy_c.reshape(UC, B, H).transpose(1, 0, 2)
    return out
